# revision 38
# baseline (speedup 1.0000x reference)
"""Multi-head attention (B=1, L=4096, D=1024, H=16, d_k=64) on 8 TRN2 NeuronCores.

Sharding: head/tensor parallel. Core c owns heads 2c, 2c+1 (128 dmodel dims):
its slices of the QKV projection weights, full-L attention for its 2 heads,
and the per-head partial O-projections for its 128-dim slice. The host gather
divides each head's partial by its softmax denominator and sums the 8 cores'
partials (the allreduce of row-sharded tensor parallelism, done at gather).

Design notes (what made this fast, ~1.5x over the f32r baseline):
- ALL matmuls fp16: f32r lowers to fp32_mode=HIGH, which power-throttled the
  PE clock (HAM K=4/8 -> 1.2 GHz) and disabled fast weight load. fp16 runs
  mostly at 2.4 GHz with FWL-hidden LDWEIGHTS.
- Both heads' score matmuls live in ONE PSUM tile [128, 2, GK*QC], so the
  paired 64x128 row-tile matmuls (T0/T8, auto tile_position from the QT/KT
  partition halves) become ready together and co-issue, streaming the two
  array halves concurrently (2x on scores; d_k=64 only fills half the
  contraction lanes otherwise).
- One fused exp per group: a single ACT instruction over the paired tile
  amortizes the ACT engine's ~352-cycle per-instruction overhead. exp on the
  scalar engine is the hard floor (~250us busy); everything else is arranged
  to keep it fed.
- AV stationaries are [V_h0 | ones64] / [ones64 | V_h1] per k-tile, so the
  softmax denominators fall out of the AV matmuls (in the otherwise-idle
  array columns) and the two heads' AV blocks land pre-stacked for the
  O-projection. Denominators ship to the host; no on-device normalize.
- Flat software-pipelined stream over (q-chunk, group) steps: AV matmuls lag
  scores by LAG groups, evac + per-head O-projection of q-chunk i drain
  inside q-chunk i+1's steps. The in-order PE queue never idles >2us: any
  >3.4us gap drops the HAM clock and the bursty steady state cannot re-warm.
- Only the K projection precedes the stream; Q is projected per q-chunk and
  V per k-tile (directly into [keys, dims] layout, no transposes) inside the
  early steps, with accumulators borrowing momentarily-idle PSUM banks.
  PSUM budget: scores 4 + AV accumulators 2 + O-proj 2 = 8 banks exactly.
- Output DMAs ride the gpsimd engine's queue so they never head-of-line
  block the input stream; evacuation copies alternate scalar/vector.
"""
import sys
if '/opt/trn_rl_repo' not in sys.path:
    sys.path.insert(0, '/opt/trn_rl_repo')

import numpy as np
from contextlib import ExitStack

import concourse.bass as bass
import concourse.tile as tile
from concourse import bacc, mybir
from concourse._compat import with_exitstack
from concourse.bass_utils import run_bass_kernel_spmd

F32 = mybir.dt.float32
F16 = mybir.dt.float16
AF = mybir.ActivationFunctionType

N_CORES = 8
L = 4096
D = 1024
QC = 512                 # q-chunk (free dim of score/AV/O matmuls)
NQC = L // QC            # 8
NKT = L // 128           # 32 k-tiles
NDT = D // 128           # 8 dmodel tiles
GK = 1                   # k-tiles per exp group ([128, 2, GK*QC] paired tiles)
SCALE = 0.125            # 1/sqrt(d_k)


def _groups():
    """Split NKT k-tiles into groups of GK (last group ragged)."""
    gs, k0 = [], 0
    while k0 < NKT:
        nk = min(GK, NKT - k0)
        gs.append((k0, nk))
        k0 += nk
    return gs


@with_exitstack
def _mha_core_kernel(ctx, tc, outs, ins, reps=1):
    sb = ctx.enter_context(tc.tile_pool(name="sb", bufs=1))
    for _rep in range(reps):
        _mha_body(tc, sb, outs, ins)


def _mha_body(tc, sb, outs, ins):
    nc = tc.nc
    outT0, outT1, dns = outs   # [NDT, NQC, 128, QC] fp16 x2, [NQC, 2, QC] fp16
    qTb, kT, vTb, wq, wk, wv, wo = ins
    # kT:  [D, L] fp16 transposed keys
    # qTb: [NQC, NDT, 128, QC] fp16 per-q-chunk blocks
    # vTb: [NKT, NDT, 128, 128] fp16 per-k-tile blocks
    # wq/wk/wv: [128, NDT, 128] fp16 (lhsT per ktile)
    # wo:       [128, NDT, 128] fp16 (lhsT per mtile; rows = local dims)

    # ---- weights (single contiguous DMAs) ----
    wq_s = sb.tile([128, NDT, 128], F16, tag="wq")
    wk_s = sb.tile([128, NDT, 128], F16, tag="wk")
    wv_s = sb.tile([128, NDT, 128], F16, tag="wv")
    wo_s = sb.tile([128, NDT, 128], F16, tag="wo")
    for w_s, w_d in ((wq_s, wq), (wk_s, wk), (wv_s, wv), (wo_s, wo)):
        nc.sync.dma_start(w_s[:], w_d[:])

    # ---- persistent activations (all fp16) ----
    QT_s = sb.tile([128, L], F16, tag="QT")
    KT_s = sb.tile([128, L], F16, tag="KT")
    # Vaug per ktile: cols [0:64]=V_h0, [64:128]=1.0  (head0 aug = [V0|1])
    #                 cols [128:192]=1.0, [192:256]=V_h1 (head1 aug = [1|V1])
    # => AV_h0 lands in psum rows 0:64 with d0 broadcast in rows 64:128;
    #    AV_h1 lands in rows 64:128 with d1 broadcast in rows 0:64.
    Vaug = sb.tile([128, NKT, 256], F16, tag="Vaug")
    nc.vector.memset(Vaug[:, :, 64:192], 1.0)

    # ========== phase 1: K projection (t-major 1 MiB streams) ==========
    # Only K must be fully projected before scores can start; Q is projected
    # per q-chunk and V per k-tile inside the phase-2 stream below.
    with (
        tc.tile_pool(name="xblk", bufs=3) as xblk,
        tc.tile_pool(name="pacc", bufs=1, space="PSUM") as pacc,
    ):
        accs = [pacc.tile([128, QC], F32, tag=f"acc{qc}", name=f"acc{qc}")
                for qc in range(NQC)]
        for t in range(NDT):
            blk = xblk.tile([128, L], F16, tag="blk")
            nc.sync.dma_start(blk[:], kT[t * 128:(t + 1) * 128, :])
            for qc in range(NQC):
                nc.tensor.matmul(accs[qc][:], wk_s[:, t, :],
                                 blk[:, qc * QC:(qc + 1) * QC],
                                 start=(t == 0), stop=(t == NDT - 1))
        for qc in range(NQC):
            if qc % 2 == 0:
                nc.scalar.copy(KT_s[:, qc * QC:(qc + 1) * QC], accs[qc][:])
            else:
                nc.vector.tensor_copy(KT_s[:, qc * QC:(qc + 1) * QC],
                                      accs[qc][:])

    # ====== phase 2: flat software-pipelined attention + O-proj stream ======
    # One flat stream of (qc, group) steps. At step p we emit the scores+exp
    # of step p and the AV matmuls of step p-LAG; a q-chunk's normalize and
    # O-projection are spread over the steps after its last AV. This keeps
    # the in-order PE queue dense across q-chunk boundaries: any >3.4us PE
    # gap drops the HAM clock to 1.2 GHz and the bursty steady state can
    # never re-warm it. The swapped denominators are matmul'd into the dead
    # halves of the ot banks so no extra PSUM bank is needed.
    groups = _groups()
    ngroups = len(groups)
    LAG = 12
    VPS = 1                  # V k-tiles projected per early step
    with (
        tc.tile_pool(name="pst", bufs=3, space="PSUM") as pst,
        tc.tile_pool(name="pot0", bufs=1, space="PSUM") as pot0,
        tc.tile_pool(name="pot1", bufs=1, space="PSUM") as pot1,
        tc.tile_pool(name="pat", bufs=LAG + 2) as pat,
        tc.tile_pool(name="psm", bufs=2) as psm,
        tc.tile_pool(name="poc", bufs=6) as poc,
        tc.tile_pool(name="qblk", bufs=3) as qblk,
        tc.tile_pool(name="vblk", bufs=3) as vblk,
    ):
        def emit_qproj(qcb):
            qb = qblk.tile([128, NDT, QC], F16, tag="qb")
            nc.gpsimd.dma_start(qb[:], qTb[qcb])
            qacc = pot0.tile([128, QC], F32, tag="ot0", name="qacc")
            for t in range(NDT):
                nc.tensor.matmul(qacc[:], wq_s[:, t, :], qb[:, t, :],
                                 start=(t == 0), stop=(t == NDT - 1))
            if qcb < 2:
                nc.scalar.copy(QT_s[:, qcb * QC:(qcb + 1) * QC], qacc[:])
            else:
                nc.vector.tensor_copy(QT_s[:, qcb * QC:(qcb + 1) * QC], qacc[:])

        def emit_vdirect(rt):
            # V projected straight into [keys, dims] layout: stationary is
            # the input block (contract over dmodel tile), moving is wv.
            vb = vblk.tile([128, NDT, 128], F16, tag="vb")
            nc.gpsimd.dma_start(vb[:], vTb[rt])
            vacc = pst.tile([128, 128], F32, tag="st", name="vacc")
            for t in range(NDT):
                nc.tensor.matmul(vacc[:], vb[:, t, :], wv_s[:, t, :],
                                 start=(t == 0), stop=(t == NDT - 1))
            nc.vector.tensor_copy(Vaug[:, rt, 0:64], vacc[:, 0:64])
            nc.vector.tensor_copy(Vaug[:, rt, 192:256], vacc[:, 64:128])
        def emit_scores_act(qc, k0, nk):
            # Both heads' scores go into ONE PSUM tile so the paired 64x128
            # row-tile matmuls (T0/T8) become ready together and co-issue,
            # streaming concurrently through the two array halves.
            q0, q1 = qc * QC, (qc + 1) * QC
            st = pst.tile([128, 2, GK * QC], F32, tag="st", name="st")
            for j in range(nk):
                ks = slice((k0 + j) * 128, (k0 + j + 1) * 128)
                js = slice(j * QC, (j + 1) * QC)
                nc.tensor.matmul(st[:, 0, js], KT_s[0:64, ks],
                                 QT_s[0:64, q0:q1], start=True, stop=True)
                nc.tensor.matmul(st[:, 1, js], KT_s[64:128, ks],
                                 QT_s[64:128, q0:q1], start=True, stop=True)
            gs = slice(0, nk * QC)
            at = pat.tile([128, 2, GK * QC], F16, tag="at")
            nc.scalar.activation(at[:, :, gs], st[:, :, gs], AF.Exp, scale=SCALE)
            return at

        def emit_av(ot0, ot1, at, k0, nk):
            for j in range(nk):
                kt = k0 + j
                js = slice(j * QC, (j + 1) * QC)
                nc.tensor.matmul(ot0[:], Vaug[:, kt, 0:128], at[:, 0, js],
                                 start=(kt == 0), stop=(kt == NKT - 1))
                nc.tensor.matmul(ot1[:], Vaug[:, kt, 128:256], at[:, 1, js],
                                 start=(kt == 0), stop=(kt == NKT - 1))

        def emit_evac(pv):
            # Evacuate the AV accumulators to SBUF (fp16) and ship the
            # denominator rows; normalization happens on the host at gather.
            ot0, ot1, qcp = pv["ot0"], pv["ot1"], pv["qc"]
            e0 = psm.tile([128, QC], F16, tag="e0")
            nc.vector.tensor_copy(e0[:], ot0[:])
            e1 = psm.tile([128, QC], F16, tag="e1")
            nc.vector.tensor_copy(e1[:], ot1[:])
            nc.gpsimd.dma_start(dns[qcp, 0, :], e1[0:1, :])    # d1
            nc.gpsimd.dma_start(dns[qcp, 1, :], e0[64:65, :])  # d0
            pv["e0"], pv["e1"] = e0, e1

        def emit_oproj(pv, mts):
            # Per-head unnormalized O-projection partials; the two heads'
            # contract-64 matmuls are complementary 64x128 row-tiles (T0/T8).
            e0, e1, qcp = pv["e0"], pv["e1"], pv["qc"]
            for mt in mts:
                op0 = pot0.tile([128, QC], F32, tag="ot0", name="op0")
                nc.tensor.matmul(op0[:], wo_s[0:64, mt, :], e0[0:64, :],
                                 start=True, stop=True)
                op1 = pot1.tile([128, QC], F32, tag="ot1", name="op1")
                nc.tensor.matmul(op1[:], wo_s[64:128, mt, :], e1[64:128, :],
                                 start=True, stop=True)
                oc0 = poc.tile([128, QC], F16, tag="oc0")
                nc.vector.tensor_copy(oc0[:], op0[:])
                nc.gpsimd.dma_start(outT0[mt, qcp, :, :], oc0[:])
                oc1 = poc.tile([128, QC], F16, tag="oc1")
                if tailmode[0]:
                    nc.scalar.copy(oc1[:], op1[:])
                else:
                    nc.vector.tensor_copy(oc1[:], op1[:])
                nc.gpsimd.dma_start(outT1[mt, qcp, :, :], oc1[:])

        flat = [(qc, gi) for qc in range(NQC) for gi in range(ngroups)]
        nsteps = len(flat)
        drain_hw = [0, 0]    # [target, next-to-drain] high-water marks
        tailmode = [False]
        state = {}          # qc -> {"ot0","ot1","ats",...}
        pending = {}        # qc whose normalize/oproj is being drained

        def drain(p):
            """Emit the deferred work whose position is p (p < nsteps+LAG)."""
            if p < 0 or p >= nsteps:
                return
            qc, gi = flat[p]
            sv = state[qc]
            if gi == 0:
                # ot banks allocated here (in emission order after any vacc
                # tiles sharing the same pool slots)
                sv["ot0"] = pot0.tile([128, QC], F32, tag="ot0", name="ot0")
                sv["ot1"] = pot1.tile([128, QC], F32, tag="ot1", name="ot1")
            emit_av(sv["ot0"], sv["ot1"], sv["ats"][gi], *groups[gi])
            sv["ats"][gi] = None
            if gi == ngroups - 1:
                pending[qc] = 0     # start output drain for this qc

        def drain_norm():
            for qc in list(pending):
                sv = state[qc]
                step = pending[qc]
                if step == 0:
                    emit_evac(sv)
                else:
                    emit_oproj(sv, [2 * (step - 1), 2 * (step - 1) + 1])
                if step == 4:
                    del pending[qc]
                    del state[qc]
                else:
                    pending[qc] = step + 1

        emit_qproj(0)
        emit_qproj(1)
        for p in range(nsteps + LAG):
            if p < nsteps:
                qc, gi = flat[p]
                if gi == 0:
                    state[qc] = {"qc": qc, "ats": [None] * ngroups}
                state[qc]["ats"][gi] = emit_scores_act(qc, *groups[gi])
                if p < NKT // VPS:
                    for rt in range(p * VPS, (p + 1) * VPS):
                        emit_vdirect(rt)
                if gi == 8 and qc + 2 < NQC:
                    emit_qproj(qc + 2)
            drain_hw[0] = max(drain_hw[0], p - LAG + 1)
            if p >= nsteps:
                drain_hw[0] = max(drain_hw[0], drain_hw[1] + 4)
            while drain_hw[1] < min(drain_hw[0], nsteps):
                drain(drain_hw[1])
                drain_hw[1] += 1
            drain_norm()
        # tail: finish the last q-chunk's evac/O-proj (scalar engine is idle
        # after the last exp, so it takes half the output casts)
        tailmode[0] = True
        while pending:
            drain_norm()


_PROGRAM = None


def _declare_io(nc):
    mk = lambda n, s, kind, dt=F16: nc.dram_tensor(n, list(s), dt, kind=kind).ap()
    blk4 = (NDT, NQC, 128, QC)
    ins = [mk("qT", (NQC, 128, NDT, QC), "ExternalInput"),
           mk("kT", (D, L), "ExternalInput"),
           mk("vT", (NKT, 128, NDT, 128), "ExternalInput"),
           mk("wq", (128, NDT, 128), "ExternalInput"),
           mk("wk", (128, NDT, 128), "ExternalInput"),
           mk("wv", (128, NDT, 128), "ExternalInput"),
           mk("wo", (128, NDT, 128), "ExternalInput")]
    outs = [mk("outT0", blk4, "ExternalOutput"),
            mk("outT1", blk4, "ExternalOutput"),
            mk("dns", (NQC, 2, QC), "ExternalOutput")]
    return ins, outs


def _build_program(reps=1):
    global _PROGRAM
    if _PROGRAM is not None and reps == 1:
        return _PROGRAM
    nc = bacc.Bacc("TRN2", target_bir_lowering=False, debug=False,
                   num_devices=N_CORES)
    ins, outs = _declare_io(nc)
    with tile.TileContext(nc) as tc:
        _mha_core_kernel(tc, outs, ins, reps=reps)
    nc.compile()
    if reps == 1:
        _PROGRAM = nc
    return nc


def _tile_T(x):
    """[L, D] -> transposed [D, L] contiguous fp16."""
    return np.ascontiguousarray(x.T.astype(np.float16))


def _tile_w(w_slice):
    """[128, D] (rows = this core's dims) -> lhsT layout [128, NDT, 128]."""
    # lhsT[p, t, m] = w_slice[m, t*128+p]
    return np.ascontiguousarray(
        w_slice.reshape(128, NDT, 128).transpose(2, 1, 0).astype(np.float16))


def make_in_maps(query, key, value, w_q, w_k, w_v, w_o):
    # qT as per-q-chunk blocks [NQC, 128, NDT, QC]; vT as per-k-tile blocks
    # [NKT, 128, NDT, 128]; kT stays [D, L].
    qT = np.ascontiguousarray(
        _tile_T(query.reshape(L, D)).reshape(NDT, 128, NQC, QC)
        .transpose(2, 1, 0, 3))
    kT = _tile_T(key.reshape(L, D))
    vT = np.ascontiguousarray(
        _tile_T(value.reshape(L, D)).reshape(NDT, 128, NKT, 128)
        .transpose(2, 1, 0, 3))
    in_maps = []
    for c in range(N_CORES):
        sl = slice(c * 128, (c + 1) * 128)
        # O-proj lhsT: wo_t[d, t, m] = w_o[t*128+m, c*128+d]
        wo_t = np.ascontiguousarray(
            w_o[:, sl].reshape(NDT, 128, 128).transpose(2, 0, 1).astype(np.float16))
        in_maps.append({
            "qT": qT, "kT": kT, "vT": vT,
            "wq": _tile_w(w_q[sl]),
            "wk": _tile_w(w_k[sl]),
            "wv": _tile_w(w_v[sl]),
            "wo": wo_t,
        })
    return in_maps


def gather_out(results):
    """Normalize per-head partials by their softmax denominators, sum the
    per-core partials, and restore [1, L, D]."""
    acc = None
    for c in range(N_CORES):
        r = results[c]
        dns = r["dns"].astype(np.float32)           # [NQC, 2, QC]
        rec1 = 1.0 / dns[:, 0, :]                   # head 1 denominators
        rec0 = 1.0 / dns[:, 1, :]                   # head 0 denominators
        # outT*[t, qc, p, j] scaled per (qc, j)
        part = (r["outT0"].astype(np.float32) * rec0[None, :, None, :]
                + r["outT1"].astype(np.float32) * rec1[None, :, None, :])
        acc = part if acc is None else acc + part
    # acc[t, qc, p, j] = out.T[t*128+p, qc*512+j] = out[qc*512+j, t*128+p]
    out = acc.transpose(1, 3, 0, 2).reshape(L, D)
    return np.ascontiguousarray(out).reshape(1, L, D)


def run(in_maps, trace=False):
    nc = _build_program()
    return run_bass_kernel_spmd(nc, in_maps, core_ids=list(range(N_CORES)),
                                trace=trace)


def kernel(query, key, value, w_q, w_k, w_v, w_o):
    query = np.asarray(query, dtype=np.float32)
    key = np.asarray(key, dtype=np.float32)
    value = np.asarray(value, dtype=np.float32)
    w_q = np.asarray(w_q, dtype=np.float32)
    w_k = np.asarray(w_k, dtype=np.float32)
    w_v = np.asarray(w_v, dtype=np.float32)
    w_o = np.asarray(w_o, dtype=np.float32)

    res = run(make_in_maps(query, key, value, w_q, w_k, w_v, w_o))
    return gather_out(res.results)


# revision 39
# speedup vs baseline: 1.2421x; 1.2421x over previous
"""Multi-head attention (B=1, L=4096, D=1024, H=16, d_k=64) on 8 TRN2 NeuronCores.

Sharding: head/tensor parallel. Core c owns heads 2c, 2c+1 (128 dmodel dims):
its slices of the QKV projection weights, full-L attention for its 2 heads,
and the per-head partial O-projections for its 128-dim slice. The host gather
divides each head's partial by its softmax denominator and sums the 8 cores'
partials (the allreduce of row-sharded tensor parallelism, done at gather).

Design notes (what made this fast, ~1.5x over the f32r baseline):
- ALL matmuls fp16: f32r lowers to fp32_mode=HIGH, which power-throttled the
  PE clock (HAM K=4/8 -> 1.2 GHz) and disabled fast weight load. fp16 runs
  mostly at 2.4 GHz with FWL-hidden LDWEIGHTS.
- Both heads' score matmuls live in ONE PSUM tile [128, 2, GK*QC], so the
  paired 64x128 row-tile matmuls (T0/T8, auto tile_position from the QT/KT
  partition halves) become ready together and co-issue, streaming the two
  array halves concurrently (2x on scores; d_k=64 only fills half the
  contraction lanes otherwise).
- One fused exp per group: a single ACT instruction over the paired tile
  amortizes the ACT engine's ~352-cycle per-instruction overhead. exp on the
  scalar engine is the hard floor (~250us busy); everything else is arranged
  to keep it fed.
- AV stationaries are [V_h0 | ones64] / [ones64 | V_h1] per k-tile, so the
  softmax denominators fall out of the AV matmuls (in the otherwise-idle
  array columns) and the two heads' AV blocks land pre-stacked for the
  O-projection. Denominators ship to the host; no on-device normalize.
- Flat software-pipelined stream over (q-chunk, group) steps: AV matmuls lag
  scores by LAG groups, evac + per-head O-projection of q-chunk i drain
  inside q-chunk i+1's steps. The in-order PE queue never idles >2us: any
  >3.4us gap drops the HAM clock and the bursty steady state cannot re-warm.
- Only the K projection precedes the stream; Q is projected per q-chunk and
  V per k-tile (directly into [keys, dims] layout, no transposes) inside the
  early steps, with accumulators borrowing momentarily-idle PSUM banks.
  PSUM budget: scores 4 + AV accumulators 2 + O-proj 2 = 8 banks exactly.
- Output DMAs ride the gpsimd engine's queue so they never head-of-line
  block the input stream; evacuation copies alternate scalar/vector.
"""
import sys
if '/opt/trn_rl_repo' not in sys.path:
    sys.path.insert(0, '/opt/trn_rl_repo')

import numpy as np
from contextlib import ExitStack

import concourse.bass as bass
import concourse.tile as tile
from concourse import bacc, mybir
from concourse._compat import with_exitstack
from concourse.bass_utils import run_bass_kernel_spmd

F32 = mybir.dt.float32
F16 = mybir.dt.float16
AF = mybir.ActivationFunctionType

N_CORES = 8
L = 4096
D = 1024
QC = 512                 # q-chunk (free dim of score/AV/O matmuls)
NQC = L // QC            # 8
NKT = L // 128           # 32 k-tiles
NDT = D // 128           # 8 dmodel tiles
GK = 1                   # k-tiles per exp group ([128, 2, GK*QC] paired tiles)
SCALE = 0.125            # 1/sqrt(d_k)


def _groups():
    """Split NKT k-tiles into groups of GK (last group ragged)."""
    gs, k0 = [], 0
    while k0 < NKT:
        nk = min(GK, NKT - k0)
        gs.append((k0, nk))
        k0 += nk
    return gs


@with_exitstack
def _mha_core_kernel(ctx, tc, outs, ins, reps=1):
    sb = ctx.enter_context(tc.tile_pool(name="sb", bufs=1))
    for _rep in range(reps):
        _mha_body(tc, sb, outs, ins)


def _mha_body(tc, sb, outs, ins):
    nc = tc.nc
    outT0, outT1, dns = outs   # [NDT, NQC, 128, QC] fp16 x2, [NQC, 2, QC] fp16
    qTb, kT, vTb, wq, wk, wv, wo = ins
    # kT:  [D, L] fp16 transposed keys
    # qTb: [NQC, NDT, 128, QC] fp16 per-q-chunk blocks
    # vTb: [NKT, NDT, 128, 128] fp16 per-k-tile blocks
    # wq/wk/wv: [128, NDT, 128] fp16 (lhsT per ktile)
    # wo:       [128, NDT, 128] fp16 (lhsT per mtile; rows = local dims)

    # ---- weights (single contiguous DMAs) ----
    wq_s = sb.tile([128, NDT, 128], F16, tag="wq")
    wk_s = sb.tile([128, NDT, 128], F16, tag="wk")
    wv_s = sb.tile([128, NDT, 128], F16, tag="wv")
    wo_s = sb.tile([128, NDT, 128], F16, tag="wo")
    for w_s, w_d in ((wq_s, wq), (wk_s, wk), (wv_s, wv), (wo_s, wo)):
        nc.sync.dma_start(w_s[:], w_d[:])

    # ---- persistent activations (all fp16) ----
    QT_s = sb.tile([128, L], F16, tag="QT")
    KT_s = sb.tile([128, L], F16, tag="KT")
    # Vaug per ktile: cols [0:64]=V_h0, [64:128]=1.0  (head0 aug = [V0|1])
    #                 cols [128:192]=1.0, [192:256]=V_h1 (head1 aug = [1|V1])
    # => AV_h0 lands in psum rows 0:64 with d0 broadcast in rows 64:128;
    #    AV_h1 lands in rows 64:128 with d1 broadcast in rows 0:64.
    Vaug = sb.tile([128, NKT, 256], F16, tag="Vaug")
    nc.vector.memset(Vaug[:, :, 64:192], 1.0)

    # ========== phase 1: K projection (t-major 1 MiB streams) ==========
    # Only K must be fully projected before scores can start; Q is projected
    # per q-chunk and V per k-tile inside the phase-2 stream below.
    with (
        tc.tile_pool(name="xblk", bufs=3) as xblk,
        tc.tile_pool(name="pacc", bufs=1, space="PSUM") as pacc,
    ):
        accs = [pacc.tile([128, QC], F32, tag=f"acc{qc}", name=f"acc{qc}")
                for qc in range(NQC)]
        for t in range(NDT):
            blk = xblk.tile([128, L], F16, tag="blk")
            nc.sync.dma_start(blk[:], kT[t * 128:(t + 1) * 128, :])
            for qc in range(NQC):
                nc.tensor.matmul(accs[qc][:], wk_s[:, t, :],
                                 blk[:, qc * QC:(qc + 1) * QC],
                                 start=(t == 0), stop=(t == NDT - 1))
        for qc in range(NQC):
            if qc % 2 == 0:
                nc.scalar.copy(KT_s[:, qc * QC:(qc + 1) * QC], accs[qc][:])
            else:
                nc.vector.tensor_copy(KT_s[:, qc * QC:(qc + 1) * QC],
                                      accs[qc][:])

    # ====== phase 2: flat software-pipelined attention + O-proj stream ======
    # One flat stream of (qc, group) steps. At step p we emit the scores+exp
    # of step p and the AV matmuls of step p-LAG; a q-chunk's normalize and
    # O-projection are spread over the steps after its last AV. This keeps
    # the in-order PE queue dense across q-chunk boundaries: any >3.4us PE
    # gap drops the HAM clock to 1.2 GHz and the bursty steady state can
    # never re-warm it. The swapped denominators are matmul'd into the dead
    # halves of the ot banks so no extra PSUM bank is needed.
    groups = _groups()
    ngroups = len(groups)
    LAG = 12
    VPS = 1                  # V k-tiles projected per early step
    with (
        tc.tile_pool(name="pst", bufs=3, space="PSUM") as pst,
        tc.tile_pool(name="pot0", bufs=1, space="PSUM") as pot0,
        tc.tile_pool(name="pot1", bufs=1, space="PSUM") as pot1,
        tc.tile_pool(name="pat", bufs=LAG + 2) as pat,
        tc.tile_pool(name="psm", bufs=2) as psm,
        tc.tile_pool(name="poc", bufs=6) as poc,
        tc.tile_pool(name="qblk", bufs=3) as qblk,
        tc.tile_pool(name="vblk", bufs=3) as vblk,
    ):
        def emit_qproj(qcb):
            qb = qblk.tile([128, NDT, QC], F16, tag="qb")
            nc.sync.dma_start(qb[:], qTb[qcb])
            qacc = pot0.tile([128, QC], F32, tag="ot0", name="qacc")
            for t in range(NDT):
                nc.tensor.matmul(qacc[:], wq_s[:, t, :], qb[:, t, :],
                                 start=(t == 0), stop=(t == NDT - 1))
            if qcb < 2:
                nc.scalar.copy(QT_s[:, qcb * QC:(qcb + 1) * QC], qacc[:])
            else:
                nc.vector.tensor_copy(QT_s[:, qcb * QC:(qcb + 1) * QC], qacc[:])

        def emit_vdirect(rt):
            # V projected straight into [keys, dims] layout: stationary is
            # the input block (contract over dmodel tile), moving is wv.
            vb = vblk.tile([128, NDT, 128], F16, tag="vb")
            nc.sync.dma_start(vb[:], vTb[rt])
            vacc = pst.tile([128, 128], F32, tag="st", name="vacc")
            for t in range(NDT):
                nc.tensor.matmul(vacc[:], vb[:, t, :], wv_s[:, t, :],
                                 start=(t == 0), stop=(t == NDT - 1))
            nc.vector.tensor_copy(Vaug[:, rt, 0:64], vacc[:, 0:64])
            nc.vector.tensor_copy(Vaug[:, rt, 192:256], vacc[:, 64:128])
        def emit_scores_act(qc, k0, nk):
            # Both heads' scores go into ONE PSUM tile so the paired 64x128
            # row-tile matmuls (T0/T8) become ready together and co-issue,
            # streaming concurrently through the two array halves.
            q0, q1 = qc * QC, (qc + 1) * QC
            st = pst.tile([128, 2, GK * QC], F32, tag="st", name="st")
            for j in range(nk):
                ks = slice((k0 + j) * 128, (k0 + j + 1) * 128)
                js = slice(j * QC, (j + 1) * QC)
                nc.tensor.matmul(st[:, 0, js], KT_s[0:64, ks],
                                 QT_s[0:64, q0:q1], start=True, stop=True)
                nc.tensor.matmul(st[:, 1, js], KT_s[64:128, ks],
                                 QT_s[64:128, q0:q1], start=True, stop=True)
            gs = slice(0, nk * QC)
            at = pat.tile([128, 2, GK * QC], F16, tag="at")
            nc.scalar.activation(at[:, :, gs], st[:, :, gs], AF.Exp, scale=SCALE)
            return at

        def emit_av(ot0, ot1, at, k0, nk):
            for j in range(nk):
                kt = k0 + j
                js = slice(j * QC, (j + 1) * QC)
                nc.tensor.matmul(ot0[:], Vaug[:, kt, 0:128], at[:, 0, js],
                                 start=(kt == 0), stop=(kt == NKT - 1))
                nc.tensor.matmul(ot1[:], Vaug[:, kt, 128:256], at[:, 1, js],
                                 start=(kt == 0), stop=(kt == NKT - 1))

        def emit_evac(pv):
            # Evacuate the AV accumulators to SBUF (fp16) and ship the
            # denominator rows; normalization happens on the host at gather.
            ot0, ot1, qcp = pv["ot0"], pv["ot1"], pv["qc"]
            e0 = psm.tile([128, QC], F16, tag="e0")
            nc.vector.tensor_copy(e0[:], ot0[:])
            e1 = psm.tile([128, QC], F16, tag="e1")
            nc.vector.tensor_copy(e1[:], ot1[:])
            nc.gpsimd.dma_start(dns[qcp, 0, :], e1[0:1, :])    # d1
            nc.gpsimd.dma_start(dns[qcp, 1, :], e0[64:65, :])  # d0
            pv["e0"], pv["e1"] = e0, e1

        def emit_oproj(pv, mts):
            # Per-head unnormalized O-projection partials; the two heads'
            # contract-64 matmuls are complementary 64x128 row-tiles (T0/T8).
            e0, e1, qcp = pv["e0"], pv["e1"], pv["qc"]
            for mt in mts:
                op0 = pot0.tile([128, QC], F32, tag="ot0", name="op0")
                nc.tensor.matmul(op0[:], wo_s[0:64, mt, :], e0[0:64, :],
                                 start=True, stop=True)
                op1 = pot1.tile([128, QC], F32, tag="ot1", name="op1")
                nc.tensor.matmul(op1[:], wo_s[64:128, mt, :], e1[64:128, :],
                                 start=True, stop=True)
                oc0 = poc.tile([128, QC], F16, tag="oc0")
                nc.vector.tensor_copy(oc0[:], op0[:])
                nc.gpsimd.dma_start(outT0[mt, qcp, :, :], oc0[:])
                oc1 = poc.tile([128, QC], F16, tag="oc1")
                if tailmode[0]:
                    nc.scalar.copy(oc1[:], op1[:])
                else:
                    nc.vector.tensor_copy(oc1[:], op1[:])
                nc.gpsimd.dma_start(outT1[mt, qcp, :, :], oc1[:])

        flat = [(qc, gi) for qc in range(NQC) for gi in range(ngroups)]
        nsteps = len(flat)
        drain_hw = [0, 0]    # [target, next-to-drain] high-water marks
        tailmode = [False]
        state = {}          # qc -> {"ot0","ot1","ats",...}
        pending = {}        # qc whose normalize/oproj is being drained

        def drain(p):
            """Emit the deferred work whose position is p (p < nsteps+LAG)."""
            if p < 0 or p >= nsteps:
                return
            qc, gi = flat[p]
            sv = state[qc]
            if gi == 0:
                # ot banks allocated here (in emission order after any vacc
                # tiles sharing the same pool slots)
                sv["ot0"] = pot0.tile([128, QC], F32, tag="ot0", name="ot0")
                sv["ot1"] = pot1.tile([128, QC], F32, tag="ot1", name="ot1")
            emit_av(sv["ot0"], sv["ot1"], sv["ats"][gi], *groups[gi])
            sv["ats"][gi] = None
            if gi == ngroups - 1:
                pending[qc] = 0     # start output drain for this qc

        def drain_norm():
            for qc in list(pending):
                sv = state[qc]
                step = pending[qc]
                if step == 0:
                    emit_evac(sv)
                else:
                    emit_oproj(sv, [2 * (step - 1), 2 * (step - 1) + 1])
                if step == 4:
                    del pending[qc]
                    del state[qc]
                else:
                    pending[qc] = step + 1

        emit_qproj(0)
        emit_qproj(1)
        for p in range(nsteps + LAG):
            if p < nsteps:
                qc, gi = flat[p]
                if gi == 0:
                    state[qc] = {"qc": qc, "ats": [None] * ngroups}
                state[qc]["ats"][gi] = emit_scores_act(qc, *groups[gi])
                if p < NKT // VPS:
                    for rt in range(p * VPS, (p + 1) * VPS):
                        emit_vdirect(rt)
                if gi == 8 and qc + 2 < NQC:
                    emit_qproj(qc + 2)
            drain_hw[0] = max(drain_hw[0], p - LAG + 1)
            if p >= nsteps:
                drain_hw[0] = max(drain_hw[0], drain_hw[1] + 4)
            while drain_hw[1] < min(drain_hw[0], nsteps):
                drain(drain_hw[1])
                drain_hw[1] += 1
            drain_norm()
        # tail: finish the last q-chunk's evac/O-proj (scalar engine is idle
        # after the last exp, so it takes half the output casts)
        tailmode[0] = True
        while pending:
            drain_norm()


_PROGRAM = None


def _declare_io(nc):
    mk = lambda n, s, kind, dt=F16: nc.dram_tensor(n, list(s), dt, kind=kind).ap()
    blk4 = (NDT, NQC, 128, QC)
    ins = [mk("qT", (NQC, 128, NDT, QC), "ExternalInput"),
           mk("kT", (D, L), "ExternalInput"),
           mk("vT", (NKT, 128, NDT, 128), "ExternalInput"),
           mk("wq", (128, NDT, 128), "ExternalInput"),
           mk("wk", (128, NDT, 128), "ExternalInput"),
           mk("wv", (128, NDT, 128), "ExternalInput"),
           mk("wo", (128, NDT, 128), "ExternalInput")]
    outs = [mk("outT0", blk4, "ExternalOutput"),
            mk("outT1", blk4, "ExternalOutput"),
            mk("dns", (NQC, 2, QC), "ExternalOutput")]
    return ins, outs


def _build_program(reps=1):
    global _PROGRAM
    if _PROGRAM is not None and reps == 1:
        return _PROGRAM
    nc = bacc.Bacc("TRN2", target_bir_lowering=False, debug=False,
                   num_devices=N_CORES)
    ins, outs = _declare_io(nc)
    with tile.TileContext(nc) as tc:
        _mha_core_kernel(tc, outs, ins, reps=reps)
    nc.compile()
    if reps == 1:
        _PROGRAM = nc
    return nc


def _tile_T(x):
    """[L, D] -> transposed [D, L] contiguous fp16."""
    return np.ascontiguousarray(x.T.astype(np.float16))


def _tile_w(w_slice):
    """[128, D] (rows = this core's dims) -> lhsT layout [128, NDT, 128]."""
    # lhsT[p, t, m] = w_slice[m, t*128+p]
    return np.ascontiguousarray(
        w_slice.reshape(128, NDT, 128).transpose(2, 1, 0).astype(np.float16))


def make_in_maps(query, key, value, w_q, w_k, w_v, w_o):
    # qT as per-q-chunk blocks [NQC, 128, NDT, QC]; vT as per-k-tile blocks
    # [NKT, 128, NDT, 128]; kT stays [D, L].
    qT = np.ascontiguousarray(
        _tile_T(query.reshape(L, D)).reshape(NDT, 128, NQC, QC)
        .transpose(2, 1, 0, 3))
    kT = _tile_T(key.reshape(L, D))
    vT = np.ascontiguousarray(
        _tile_T(value.reshape(L, D)).reshape(NDT, 128, NKT, 128)
        .transpose(2, 1, 0, 3))
    in_maps = []
    for c in range(N_CORES):
        sl = slice(c * 128, (c + 1) * 128)
        # O-proj lhsT: wo_t[d, t, m] = w_o[t*128+m, c*128+d]
        wo_t = np.ascontiguousarray(
            w_o[:, sl].reshape(NDT, 128, 128).transpose(2, 0, 1).astype(np.float16))
        in_maps.append({
            "qT": qT, "kT": kT, "vT": vT,
            "wq": _tile_w(w_q[sl]),
            "wk": _tile_w(w_k[sl]),
            "wv": _tile_w(w_v[sl]),
            "wo": wo_t,
        })
    return in_maps


def gather_out(results):
    """Normalize per-head partials by their softmax denominators, sum the
    per-core partials, and restore [1, L, D]."""
    acc = None
    for c in range(N_CORES):
        r = results[c]
        dns = r["dns"].astype(np.float32)           # [NQC, 2, QC]
        rec1 = 1.0 / dns[:, 0, :]                   # head 1 denominators
        rec0 = 1.0 / dns[:, 1, :]                   # head 0 denominators
        # outT*[t, qc, p, j] scaled per (qc, j)
        part = (r["outT0"].astype(np.float32) * rec0[None, :, None, :]
                + r["outT1"].astype(np.float32) * rec1[None, :, None, :])
        acc = part if acc is None else acc + part
    # acc[t, qc, p, j] = out.T[t*128+p, qc*512+j] = out[qc*512+j, t*128+p]
    out = acc.transpose(1, 3, 0, 2).reshape(L, D)
    return np.ascontiguousarray(out).reshape(1, L, D)


def run(in_maps, trace=False):
    nc = _build_program()
    return run_bass_kernel_spmd(nc, in_maps, core_ids=list(range(N_CORES)),
                                trace=trace)


def kernel(query, key, value, w_q, w_k, w_v, w_o):
    query = np.asarray(query, dtype=np.float32)
    key = np.asarray(key, dtype=np.float32)
    value = np.asarray(value, dtype=np.float32)
    w_q = np.asarray(w_q, dtype=np.float32)
    w_k = np.asarray(w_k, dtype=np.float32)
    w_v = np.asarray(w_v, dtype=np.float32)
    w_o = np.asarray(w_o, dtype=np.float32)

    res = run(make_in_maps(query, key, value, w_q, w_k, w_v, w_o))
    return gather_out(res.results)
